# revision 31
# baseline (speedup 1.0000x reference)
"""Trainium2 Bass kernel for BatchRemoveQuatDiscontinuities.

Algorithm (per (batch, joint) lane):
    d[t]    = dot(q[t], q[t-1])                (fp32, 4-wide dot)
    flip[t] = 1 if d[t] < 0 else 0             (t >= 1; flip[0] = 0)
    sigma[t] = (-1)^(sum_{s<=t} flip[s])       (cumulative sign parity)
    out[t]  = q[t] * sigma[t]

Mapping on a NeuronCore (data-parallel over batch across 8 cores):
  * One tile = one batch clip, loaded as a single fully-contiguous 1MB
    DMA: [128 partitions = t/8, free = (ts: 8, j: 64, c: 4)].  This is
    just the flat memory order of q[b], so DMA runs at the HBM roofline.
  * q[t-1]: within a partition it is a free-axis offset (-256); the
    octet boundary (ts=0) needs q[p-1, ts=7], produced by a TensorE
    matmul with an off-diagonal 0/1 matrix S into PSUM (fp32 exact).
  * prod on VectorE, 4-wide dot via two pairwise adds (c0+c1)+(c2+c3),
    written in (j, ts) order; flip indicator e = Relu(Sign(-d)) on
    ScalarE (bf16).
  * Within-octet inclusive prefix: tensor_tensor_scan with a reset mask
    (state = mask*state + e), segments of 8 per joint.  Octet-level
    exclusive prefix: strict-triangular matmul over partitions on the
    per-row totals (strided rhs slice ts=7).  total = rowpref + offs.
  * Parity: cast to int32, &1, then sigma = 1 - 2*p on ScalarE (bf16).
    GpSimd multiplies out = q * sigma (broadcast over c) - exact +/-1.
"""

import numpy as np
import ml_dtypes
from contextlib import ExitStack

import concourse.bass as bass
import concourse.bacc as bacc
import concourse.tile as tile
from concourse import mybir
from concourse.bass_utils import run_bass_kernel_spmd

B, T, J, C = 128, 1024, 64, 4
NCORES = 8
JC = J * C                      # 256 floats per t
BPC = B // NCORES               # 16 batch clips per core
TS = 8                          # t per partition (octet)
FD = TS * JC                    # tile free dim = 2048 floats
SD = J * TS                     # prefix free dim = 512 (j, ts)

FP32 = mybir.dt.float32
BF16 = mybir.dt.bfloat16
I32 = mybir.dt.int32
Alu = mybir.AluOpType
Act = mybir.ActivationFunctionType


def _ap(apx, dims):
    """AP with explicit [step, count] free dims appended to partition dim."""
    return bass.AP(
        tensor=apx.tensor, offset=apx.offset,
        ap=[list(apx.ap[0]), *[list(d) for d in dims]],
    )


def build_nc(bpc=BPC, t=T, reps=1, mode="full"):
    assert t % (128 * TS) == 0
    tpp = t // 128              # t-octets per partition per clip (1 for T=1024)
    nc = bacc.Bacc(None, target_bir_lowering=False)
    q = nc.declare_dram_parameter("q", [bpc, t, J, C], FP32, isOutput=False)
    smat = nc.declare_dram_parameter("smat", [128, 128], FP32, isOutput=False)
    pmat = nc.declare_dram_parameter("pmat", [128, 128], FP32, isOutput=False)
    out = nc.declare_dram_parameter("out", [bpc, t, J, C], FP32, isOutput=True)
    qf = q.rearrange("b t j c -> b (t j c)")
    of = out.rearrange("b t j c -> b (t j c)")

    with tile.TileContext(nc) as tc, ExitStack() as ctx:
        consts = ctx.enter_context(tc.tile_pool(name="consts", bufs=1))
        qpool = ctx.enter_context(tc.tile_pool(name="qpool", bufs=8))
        opool = ctx.enter_context(tc.tile_pool(name="opool", bufs=5))
        spool = ctx.enter_context(tc.tile_pool(name="spool", bufs=4))
        auxp = ctx.enter_context(tc.tile_pool(name="auxp", bufs=4, space="PSUM"))
        offp = ctx.enter_context(tc.tile_pool(name="offp", bufs=4, space="PSUM"))

        smatSB = consts.tile([128, 128], FP32)
        nc.sync.dma_start(out=smatSB[:, :], in_=smat[:, :])
        pmatSB = consts.tile([128, 128], FP32)
        nc.sync.dma_start(out=pmatSB[:, :], in_=pmat[:, :])
        amask = consts.tile([128, SD], FP32)
        nc.vector.memset(amask[:, :], 1.0)
        nc.vector.memset(
            amask.rearrange("p (j ts) -> p j ts", ts=TS)[:, :, 0], 0.0
        )

        def emit_body():
            for b in range(bpc):
                emit_tile(b)

        def emit_tile(b):
            qt = qpool.tile([128, FD], FP32, tag="qt")
            nc.sync.dma_start(
                out=qt[:, :],
                in_=qf[b, :].rearrange("(p x) -> p x", p=128),
            )
            o = opool.tile([128, FD], FP32, tag="o")
            if mode == "dma":
                nc.sync.dma_start(
                    out=of[b, :].rearrange("(p x) -> p x", p=128), in_=qt[:, :]
                )
                return

            # octet-boundary shift: aux[p] = qt[p-1, ts=7 chunk] (row 0 = 0)
            aux = auxp.tile([128, JC], FP32, tag="aux")
            nc.tensor.matmul(
                aux[:, :],
                lhsT=smatSB[:, :],
                rhs=qt[:, FD - JC:FD],
                start=True,
                stop=True,
            )

            # prod: o = q * q_shifted
            nc.vector.tensor_tensor(
                out=o[:, JC:FD], in0=qt[:, JC:FD], in1=qt[:, 0:FD - JC],
                op=Alu.mult,
            )
            nc.vector.tensor_tensor(
                out=o[:, 0:JC], in0=qt[:, 0:JC], in1=aux[:, :], op=Alu.mult,
            )

            # dot over c, pairwise (c0+c1)+(c2+c3); d written in (j, ts) order
            u = spool.tile([128, 2 * SD], FP32, tag="u")
            ov = o.rearrange("p (s c) -> p s c", c=C)
            uv = u.rearrange("p (s k) -> p s k", k=2)
            opairs = ov.rearrange("p s (k two) -> p s k two", k=2)
            nc.vector.tensor_tensor(
                out=uv, in0=opairs[:, :, :, 0], in1=opairs[:, :, :, 1],
                op=Alu.add,
            )
            d = spool.tile([128, SD], FP32, tag="d")  # (j, ts) layout
            u_k = u.rearrange("p (ts j k) -> p ts j k", j=J, k=2)
            nc.vector.tensor_tensor(
                out=_ap(d, [[1, TS], [TS, J]]),
                in0=u_k[:, :, :, 0],
                in1=u_k[:, :, :, 1],
                op=Alu.add,
            )

            # flip indicator e = Relu(Sign(-d)), bf16, (j, ts) layout
            sg = spool.tile([128, SD], FP32, tag="sg")
            nc.scalar.activation(sg[:, :], d[:, :], Act.Sign, scale=-1.0)
            e = spool.tile([128, SD], BF16, tag="e")
            nc.scalar.activation(e[:, :], sg[:, :], Act.Relu)
            # t=0 has no flip (also guards Sign(0) semantics)
            nc.scalar.mul(
                e.rearrange("p (j ts) -> p j ts", ts=TS)[0:1, :, 0],
                e.rearrange("p (j ts) -> p j ts", ts=TS)[0:1, :, 0],
                0.0,
            )

            # within-octet inclusive prefix PARITY (segmented xor-scan):
            # state = (mask * state) xor e  -> 0/1 running parity per joint
            rowp = spool.tile([128, SD], FP32, tag="rowp")
            nc.vector.tensor_tensor_scan(
                out=rowp[:, :], data0=amask[:, :], data1=e[:, :],
                initial=0.0, op0=Alu.mult, op1=Alu.logical_xor,
            )

            # octet-level: count of odd rows above (parity-sum via matmul)
            offs = offp.tile([128, J], FP32, tag="offs")
            nc.tensor.matmul(
                offs[:, :],
                lhsT=pmatSB[:, :],
                rhs=rowp.rearrange("p (j ts) -> p j ts", ts=TS)[:, :, 7],
                start=True,
                stop=True,
            )
            # parity of that count -> sigma_off in {+1, -1} per (p, j)
            offi = spool.tile([128, J], I32, tag="offi")
            nc.vector.tensor_copy(out=offi[:, :], in_=offs[:, :])
            offb = spool.tile([128, J], I32, tag="offb")
            nc.vector.tensor_scalar(
                out=offb[:, :], in0=offi[:, :], scalar1=1, scalar2=None,
                op0=Alu.bitwise_and,
            )
            sigo = spool.tile([128, J], BF16, tag="sigo")
            nc.scalar.activation(sigo[:, :], offb[:, :], Act.Copy,
                                 bias=1.0, scale=-2.0)
            # sigma_row in {+1, -1} from the 0/1 row parity
            sigr = spool.tile([128, SD], BF16, tag="sigr")
            nc.scalar.activation(sigr[:, :], rowp[:, :], Act.Copy,
                                 bias=1.0, scale=-2.0)
            # sigma = sigma_row * sigma_off, (j, ts) layout
            sig = spool.tile([128, SD], BF16, tag="sig")
            nc.gpsimd.tensor_tensor(
                out=sig[:, :], in0=sigr[:, :],
                in1=_ap(sigo, [[1, J], [0, TS]]),
                op=Alu.mult,
            )

            # out = q * sigma (broadcast over c), exact +/-1 multiply
            nc.gpsimd.tensor_tensor(
                out=ov,
                in0=qt.rearrange("p (s c) -> p s c", c=C),
                in1=bass.AP(tensor=sig.tensor, offset=sig.offset,
                            ap=[list(sig.ap[0]), [1, TS], [TS, J], [0, C]]),
                op=Alu.mult,
            )

            nc.sync.dma_start(
                out=of[b, :].rearrange("(p x) -> p x", p=128), in_=o[:, :]
            )

        if reps == 1:
            emit_body()
        else:
            with tc.For_i(0, reps, 1):
                emit_body()
    return nc


def make_consts():
    smat = np.eye(128, k=1, dtype=np.float32)       # S[k, m] = 1 iff m == k+1
    pmat = np.triu(np.ones((128, 128), np.float32), k=1)  # strict prefix
    return smat, pmat


def kernel(joint_rotations: np.ndarray) -> np.ndarray:
    q = np.ascontiguousarray(joint_rotations, dtype=np.float32)
    assert q.shape == (B, T, J, C)
    smat, pmat = make_consts()
    nc = build_nc()
    nc.finalize()   # run bacc passes (wait splitting, reg alloc) + freeze
    in_maps = [
        {"q": q[c * BPC:(c + 1) * BPC], "smat": smat, "pmat": pmat}
        for c in range(NCORES)
    ]
    res = run_bass_kernel_spmd(nc, in_maps, list(range(NCORES)))
    outs = [np.asarray(r["out"]) for r in res.results]
    return np.concatenate(outs, axis=0)


# revision 32
# speedup vs baseline: 1.9873x; 1.9873x over previous
"""Trainium2 Bass kernel for BatchRemoveQuatDiscontinuities.

Algorithm (per (batch, joint) lane):
    d[t]    = dot(q[t], q[t-1])                (fp32, 4-wide dot)
    flip[t] = 1 if d[t] < 0 else 0             (t >= 1; flip[0] = 0)
    sigma[t] = (-1)^(sum_{s<=t} flip[s])       (cumulative sign parity)
    out[t]  = q[t] * sigma[t]

Mapping on a NeuronCore (data-parallel over batch across 8 cores):
  * One tile = one batch clip, loaded as a single fully-contiguous 1MB
    DMA: [128 partitions = t/8, free = (ts: 8, j: 64, c: 4)].  This is
    just the flat memory order of q[b], so DMA runs at the HBM roofline.
  * q[t-1]: within a partition it is a free-axis offset (-256); the
    octet boundary (ts=0) needs q[p-1, ts=7], produced by a TensorE
    matmul with an off-diagonal 0/1 matrix S into PSUM (fp32 exact).
  * prod on VectorE, 4-wide dot via two pairwise adds (c0+c1)+(c2+c3),
    written in (j, ts) order; flip indicator e = Relu(Sign(-d)) on
    ScalarE (bf16).
  * Within-octet inclusive prefix: tensor_tensor_scan with a reset mask
    (state = mask*state + e), segments of 8 per joint.  Octet-level
    exclusive prefix: strict-triangular matmul over partitions on the
    per-row totals (strided rhs slice ts=7).  total = rowpref + offs.
  * Parity: cast to int32, &1, then sigma = 1 - 2*p on ScalarE (bf16).
    GpSimd multiplies out = q * sigma (broadcast over c) - exact +/-1.
"""

import numpy as np
import ml_dtypes
from contextlib import ExitStack

import concourse.bass as bass
import concourse.bacc as bacc
import concourse.tile as tile
from concourse import mybir
from concourse.bass_utils import run_bass_kernel_spmd

B, T, J, C = 128, 1024, 64, 4
NCORES = 8
JC = J * C                      # 256 floats per t
BPC = B // NCORES               # 16 batch clips per core
TS = 8                          # t per partition (octet)
FD = TS * JC                    # tile free dim = 2048 floats
SD = J * TS                     # prefix free dim = 512 (j, ts)

FP32 = mybir.dt.float32
BF16 = mybir.dt.bfloat16
I32 = mybir.dt.int32
Alu = mybir.AluOpType
Act = mybir.ActivationFunctionType


def _ap(apx, dims):
    """AP with explicit [step, count] free dims appended to partition dim."""
    return bass.AP(
        tensor=apx.tensor, offset=apx.offset,
        ap=[list(apx.ap[0]), *[list(d) for d in dims]],
    )


def build_nc(bpc=BPC, t=T, reps=1, mode="full"):
    assert t % (128 * TS) == 0
    tpp = t // 128              # t-octets per partition per clip (1 for T=1024)
    nc = bacc.Bacc(None, target_bir_lowering=False)
    q = nc.declare_dram_parameter("q", [bpc, t, J, C], FP32, isOutput=False)
    smat = nc.declare_dram_parameter("smat", [128, 128], FP32, isOutput=False)
    pmat = nc.declare_dram_parameter("pmat", [128, 128], FP32, isOutput=False)
    out = nc.declare_dram_parameter("out", [bpc, t, J, C], FP32, isOutput=True)
    qf = q.rearrange("b t j c -> b (t j c)")
    of = out.rearrange("b t j c -> b (t j c)")

    with tile.TileContext(nc) as tc, ExitStack() as ctx:
        consts = ctx.enter_context(tc.tile_pool(name="consts", bufs=1))
        qpool = ctx.enter_context(tc.tile_pool(name="qpool", bufs=8))
        opool = ctx.enter_context(tc.tile_pool(name="opool", bufs=5))
        spool = ctx.enter_context(tc.tile_pool(name="spool", bufs=4))
        auxp = ctx.enter_context(tc.tile_pool(name="auxp", bufs=4, space="PSUM"))
        offp = ctx.enter_context(tc.tile_pool(name="offp", bufs=4, space="PSUM"))

        smatSB = consts.tile([128, 128], FP32)
        nc.sync.dma_start(out=smatSB[:, :], in_=smat[:, :])
        pmatSB = consts.tile([128, 128], FP32)
        nc.sync.dma_start(out=pmatSB[:, :], in_=pmat[:, :])
        amask = consts.tile([128, SD], FP32)
        nc.vector.memset(amask[:, :], 1.0)
        nc.vector.memset(
            amask.rearrange("p (j ts) -> p j ts", ts=TS)[:, :, 0], 0.0
        )

        def emit_body():
            for b in range(bpc):
                emit_tile(b)

        def emit_tile(b):
            qt = qpool.tile([128, FD], FP32, tag="qt")
            nc.sync.dma_start(
                out=qt[:, :],
                in_=qf[b, :].rearrange("(p x) -> p x", p=128),
            )
            o = opool.tile([128, FD], FP32, tag="o")
            if mode == "dma":
                nc.sync.dma_start(
                    out=of[b, :].rearrange("(p x) -> p x", p=128), in_=qt[:, :]
                )
                return

            # octet-boundary shift: aux[p] = qt[p-1, ts=7 chunk] (row 0 = 0)
            aux = auxp.tile([128, JC], FP32, tag="aux")
            nc.tensor.matmul(
                aux[:, :],
                lhsT=smatSB[:, :],
                rhs=qt[:, FD - JC:FD],
                start=True,
                stop=True,
            )

            # prod: o = q * q_shifted
            nc.vector.tensor_tensor(
                out=o[:, JC:FD], in0=qt[:, JC:FD], in1=qt[:, 0:FD - JC],
                op=Alu.mult,
            )
            nc.vector.tensor_tensor(
                out=o[:, 0:JC], in0=qt[:, 0:JC], in1=aux[:, :], op=Alu.mult,
            )

            # dot over c, pairwise (c0+c1)+(c2+c3); d written in (j, ts) order
            u = spool.tile([128, 2 * SD], FP32, tag="u")
            ov = o.rearrange("p (s c) -> p s c", c=C)
            uv = u.rearrange("p (s k) -> p s k", k=2)
            opairs = ov.rearrange("p s (k two) -> p s k two", k=2)
            nc.vector.tensor_tensor(
                out=uv, in0=opairs[:, :, :, 0], in1=opairs[:, :, :, 1],
                op=Alu.add,
            )
            d = spool.tile([128, SD], FP32, tag="d")  # (j, ts) layout
            u_k = u.rearrange("p (ts j k) -> p ts j k", j=J, k=2)
            nc.vector.tensor_tensor(
                out=_ap(d, [[1, TS], [TS, J]]),
                in0=u_k[:, :, :, 0],
                in1=u_k[:, :, :, 1],
                op=Alu.add,
            )

            # flip indicator e = Relu(Sign(-d)), bf16, (j, ts) layout
            sg = spool.tile([128, SD], FP32, tag="sg")
            nc.scalar.activation(sg[:, :], d[:, :], Act.Sign, scale=-1.0)
            e = spool.tile([128, SD], BF16, tag="e")
            nc.scalar.activation(e[:, :], sg[:, :], Act.Relu)
            # t=0 has no flip (also guards Sign(0) semantics)
            nc.scalar.mul(
                e.rearrange("p (j ts) -> p j ts", ts=TS)[0:1, :, 0],
                e.rearrange("p (j ts) -> p j ts", ts=TS)[0:1, :, 0],
                0.0,
            )

            # within-octet inclusive prefix PARITY (segmented xor-scan):
            # state = (mask * state) xor e  -> 0/1 running parity per joint
            rowp = spool.tile([128, SD], FP32, tag="rowp")
            nc.vector.tensor_tensor_scan(
                out=rowp[:, :], data0=amask[:, :], data1=e[:, :],
                initial=0.0, op0=Alu.mult, op1=Alu.logical_xor,
            )

            # octet-level: count of odd rows above (parity-sum via matmul)
            offs = offp.tile([128, J], FP32, tag="offs")
            nc.tensor.matmul(
                offs[:, :],
                lhsT=pmatSB[:, :],
                rhs=rowp.rearrange("p (j ts) -> p j ts", ts=TS)[:, :, 7],
                start=True,
                stop=True,
            )
            # parity of that count -> sigma_off in {+1, -1} per (p, j)
            offi = spool.tile([128, J], I32, tag="offi")
            nc.vector.tensor_copy(out=offi[:, :], in_=offs[:, :])
            offb = spool.tile([128, J], I32, tag="offb")
            nc.vector.tensor_scalar(
                out=offb[:, :], in0=offi[:, :], scalar1=1, scalar2=None,
                op0=Alu.bitwise_and,
            )
            sigo = spool.tile([128, J], BF16, tag="sigo")
            nc.scalar.activation(sigo[:, :], offb[:, :], Act.Copy,
                                 bias=1.0, scale=-2.0)
            # sigma_row in {+1, -1} from the 0/1 row parity
            sigr = spool.tile([128, SD], BF16, tag="sigr")
            nc.scalar.activation(sigr[:, :], rowp[:, :], Act.Copy,
                                 bias=1.0, scale=-2.0)
            # sigma = sigma_row * sigma_off, (j, ts) layout
            sig = spool.tile([128, SD], BF16, tag="sig")
            sig_eng = nc.vector if mode == "nogp" else nc.gpsimd
            sig_eng.tensor_tensor(
                out=sig[:, :], in0=sigr[:, :],
                in1=_ap(sigo, [[1, J], [0, TS]]),
                op=Alu.mult,
            )

            # out = q * sigma (broadcast over c), exact +/-1 multiply
            if mode != "nogp":
                nc.gpsimd.tensor_tensor(
                    out=ov,
                    in0=qt.rearrange("p (s c) -> p s c", c=C),
                    in1=bass.AP(tensor=sig.tensor, offset=sig.offset,
                                ap=[list(sig.ap[0]), [1, TS], [TS, J], [0, C]]),
                    op=Alu.mult,
                )

            nc.sync.dma_start(
                out=of[b, :].rearrange("(p x) -> p x", p=128), in_=o[:, :]
            )

        if reps == 1:
            emit_body()
        else:
            with tc.For_i(0, reps, 1):
                emit_body()
    return nc


def make_consts():
    smat = np.eye(128, k=1, dtype=np.float32)       # S[k, m] = 1 iff m == k+1
    pmat = np.triu(np.ones((128, 128), np.float32), k=1)  # strict prefix
    return smat, pmat


def kernel(joint_rotations: np.ndarray) -> np.ndarray:
    q = np.ascontiguousarray(joint_rotations, dtype=np.float32)
    assert q.shape == (B, T, J, C)
    smat, pmat = make_consts()
    nc = build_nc()
    nc.finalize()   # run bacc passes (wait splitting, reg alloc) + freeze
    in_maps = [
        {"q": q[c * BPC:(c + 1) * BPC], "smat": smat, "pmat": pmat}
        for c in range(NCORES)
    ]
    res = run_bass_kernel_spmd(nc, in_maps, list(range(NCORES)))
    outs = [np.asarray(r["out"]) for r in res.results]
    return np.concatenate(outs, axis=0)
